# revision 30
# baseline (speedup 1.0000x reference)
# Trainium2 Bass kernel for nn_CauRecNet (2-layer residual-cell LSTM scan).
#
# Strategy: pure data-parallel over 8 NeuronCores (batch 131072 -> 16384/core).
# Per core: For_i over 16 "pair tiles" of 1024 batch rows = two 512-halves
# (A, B). Activations are feature-major ([feature, batch] in SBUF) so batch
# rides the matmul free dim (N=512). L0 (H1=64) keeps both halves STACKED on
# the partition axis; L1 slices the stacked h0 by partition range (K=64).
#
# v2 changes vs baseline:
#  - input x / cell_state transposed to feature-major via DMA xbar transpose
#    (bf16, 16x128 tiles) instead of PE transposes through PSUM; the f32->bf16
#    conversion runs on the (otherwise idle) gpsimd engine.
#  - single rotating PSUM pool (tag "G", bufs=2 x [128,2048] = all 8 banks):
#    G0 / G1A / G1B / init / head alternate bank groups, so the PE never
#    waits on the activation reads of the immediately-preceding gate group.
#  - L1 ih reads h0[0:64]/h0[64:128] partition slices (no zero-padded weights).
#  - init matmuls in bf16; pred stores issued on the ACT DMA queue so the SP
#    queue (loads + transposes) never head-of-line blocks the next tile.
#  - For_i(staggered_reset=True) to soften the per-iteration barrier.
#
# dtypes: matmuls bf16, gate activations / intermediates bf16, c-state fp32.

import numpy as np
import ml_dtypes

B, T, F = 131072, 15, 12
H1, H2, CS = 64, 128, 96
NCORES = 8
BL = B // NCORES          # 16384 rows per core
NT = 512                  # matmul free dim (one half)
NPAIR = BL // (2 * NT)    # 16 pair-tiles per core

BF16 = ml_dtypes.bfloat16

_BUILD_CACHE = {}


def _build_bass(has_gate_bias, has_vec_bias, repeat=1):
    import concourse.bacc as bacc
    import concourse.tile as tile
    from concourse import mybir
    from concourse.masks import make_identity
    from concourse.expressions_rust import smin

    f32 = mybir.dt.float32
    bf16 = mybir.dt.bfloat16
    AF = mybir.ActivationFunctionType

    nc = bacc.Bacc()

    # ---- DRAM I/O ----
    x_d = nc.dram_tensor("input_seq", [BL, T, F], f32, kind="ExternalInput")
    cs_d = nc.dram_tensor("cell_state", [BL, CS], f32, kind="ExternalInput")
    w0ih_d = nc.dram_tensor("w0pad", [128, T * 4 * H1], bf16, kind="ExternalInput")
    w0hh_d = nc.dram_tensor("w0hh_bd", [2 * H1, 4 * H1 * 2], bf16, kind="ExternalInput")
    w1ih_d = nc.dram_tensor("w1dup", [2 * H1, 4 * H2], bf16, kind="ExternalInput")
    w1hh_d = nc.dram_tensor("w1hhT", [H2, 4 * H2], bf16, kind="ExternalInput")
    fc1A_d = nc.dram_tensor("fc1_A", [CS, 2 * H1], f32, kind="ExternalInput")
    fc1B_d = nc.dram_tensor("fc1_B", [CS, 2 * H1], f32, kind="ExternalInput")
    fc2_d = nc.dram_tensor("fc2T", [CS, H2], f32, kind="ExternalInput")
    d1_d = nc.dram_tensor("d1T", [H2, H1], bf16, kind="ExternalInput")
    d2_d = nc.dram_tensor("d2T", [H1, 1], bf16, kind="ExternalInput")
    gb_d = nc.dram_tensor("gate_bias", [128, 8], f32, kind="ExternalInput")
    vb_d = nc.dram_tensor("vec_bias", [128, 4], f32, kind="ExternalInput")
    pred_d = nc.dram_tensor("pred", [BL, 1], f32, kind="ExternalOutput")

    # DRAM views ([pair, ...])
    x_view = x_d[:].rearrange("(n c p) t f -> n p c (t f)", c=8, p=128)    # [16,128,8,180]
    cs_view = cs_d[:].rearrange("(n c p) k -> n p c k", c=8, p=128)        # [16,128,8,96]
    pred_view = pred_d[:].rearrange("(n h x) o -> n h o x", h=2, x=NT)     # [16,2,1,512]

    with tile.TileContext(nc) as tc:
        import contextlib
        ctx = contextlib.ExitStack()
        with ctx:
            consts = ctx.enter_context(tc.tile_pool(name="consts", bufs=1))
            loads = ctx.enter_context(tc.tile_pool(name="loads", bufs=1))
            conv = ctx.enter_context(tc.tile_pool(name="conv", bufs=2))
            xts = ctx.enter_context(tc.tile_pool(name="xts", bufs=1))
            states = ctx.enter_context(tc.tile_pool(name="states", bufs=2))
            scratch = ctx.enter_context(tc.tile_pool(name="scratch", bufs=2))
            outp = ctx.enter_context(tc.tile_pool(name="outp", bufs=2))
            ppg = [ctx.enter_context(tc.tile_pool(name=f"ppg{s}", bufs=1,
                                                   space="PSUM"))
                   for s in range(2)]

            ident = consts.tile([128, 128], f32)
            make_identity(nc, ident)

            def load_const(name, dram, shape, dt):
                t = consts.tile(shape, dt, name=name)
                nc.sync.dma_start(out=t, in_=dram[:])
                return t

            w0pad_flat = load_const("w0pad", w0ih_d, [128, T * 4 * H1], bf16)
            w0pad = w0pad_flat.rearrange("p (t g) -> p t g", t=T)   # [128,15,256]
            w0hh = load_const("w0hh", w0hh_d, [2 * H1, 512], bf16)
            w1dup = load_const("w1dup", w1ih_d, [2 * H1, 4 * H2], bf16)
            w1hh = load_const("w1hh", w1hh_d, [H2, 512], bf16)
            fc1A = load_const("fc1A", fc1A_d, [CS, 128], f32)
            fc1B = load_const("fc1B", fc1B_d, [CS, 128], f32)
            fc2 = load_const("fc2", fc2_d, [CS, H2], f32)
            d1w = load_const("d1w", d1_d, [H2, H1], bf16)
            d2w = load_const("d2w", d2_d, [H1, 1], bf16)
            gbias = load_const("gbias", gb_d, [128, 8], f32)
            vbias = load_const("vbias", vb_d, [128, 4], f32)

            # persistent load buffers, one pair per phase: prologue fills
            # tiles 0..3; each phase computes from its pair and prefetches the
            # pair it will need next body (+4) once its reads complete.
            x_nats = [loads.tile([128, 8, T * F], f32, tag=f"x_nat{q}",
                                 name=f"x_nat{q}") for q in range(4)]
            cs_nats = [loads.tile([128, 8, CS], f32, tag=f"cs_nat{q}",
                                  name=f"cs_nat{q}") for q in range(4)]

            def prologue_loads():
                for q in range(4):
                    nc.sync.dma_start(out=x_nats[q], in_=x_view[q])
                    nc.sync.dma_start(out=cs_nats[q], in_=cs_view[q])

            def prefetch_stream(q, it):
                # clamped: the tail redundantly re-loads tile NPAIR-1
                idx = smin(it + 4, NPAIR - 1)
                nc.sync.dma_start(out=x_nats[q], in_=x_view[idx])
                nc.sync.dma_start(out=cs_nats[q], in_=cs_view[idx])

            def load_stream(s, q):
                return {"s": s, "x_nat": x_nats[q], "cs_nat": cs_nats[q]}

            def transpose_stream(st):
                # PE xbar transposes into the stream's PSUM group, then one
                # wide DVE copy converts f32 -> bf16 into SBUF.
                # xT cols 0:1024 rows = (t,f) elems 0:128 (t0..t9, batch 0:1024)
                # xT cols 1024:2048 rows = elems 52:180 (t10..t14 at 12t-52)
                s = st["s"]
                tp_x = ppg[s].tile([128, 2048], f32, tag="G", name="tp_x")
                for c in range(8):
                    nc.tensor.transpose(tp_x[:, c * 128:(c + 1) * 128],
                                        st["x_nat"][:, c, 0:128], ident)
                    nc.tensor.transpose(tp_x[:, 1024 + c * 128:1152 + c * 128],
                                        st["x_nat"][:, c, 52:180], ident)
                xT = xts.tile([128, 2048], bf16, tag=f"xT{s}", name="xT")
                nc.vector.tensor_copy(out=xT, in_=tp_x)
                tp_c = ppg[s].tile([128, 2048], f32, tag="G", name="tp_c")
                for c in range(8):
                    nc.tensor.transpose(tp_c[0:CS, c * 128:(c + 1) * 128],
                                        st["cs_nat"][:, c, :], ident)
                csT = xts.tile([CS, 1024], f32, tag=f"csT{s}", name="csT")
                nc.vector.tensor_copy(out=csT, in_=tp_c[0:CS, 0:1024])
                st["xT"], st["csT"] = xT, csT

            def init_stream(st):
                s = st["s"]
                csT = st["csT"]
                ip = ppg[s].tile([128, 2048], f32, tag="G", name="ip")
                nc.tensor.matmul(ip[:, 0:512], fc1A, csT[0:CS, 0:512],
                                 start=True, stop=False)
                nc.tensor.matmul(ip[:, 0:512], fc1B, csT[0:CS, 512:1024],
                                 start=False, stop=True)
                nc.tensor.matmul(ip[:, 512:1024], fc2, csT[0:CS, 0:512],
                                 start=True, stop=True)
                nc.tensor.matmul(ip[:, 1024:1536], fc2, csT[0:CS, 512:1024],
                                 start=True, stop=True)
                c0 = states.tile([128, NT], f32, tag=f"c0{s}", name="c0")
                c1A = states.tile([H2, NT], f32, tag=f"c1A{s}", name="c1A")
                c1B = states.tile([H2, NT], f32, tag=f"c1B{s}", name="c1B")
                if has_vec_bias:
                    nc.vector.tensor_scalar_add(c0, ip[:, 0:512], vbias[:, 0:1])
                    nc.vector.tensor_scalar_add(c1A, ip[:, 512:1024], vbias[:, 1:2])
                    nc.vector.tensor_scalar_add(c1B, ip[:, 1024:1536], vbias[:, 1:2])
                else:
                    nc.vector.tensor_copy(out=c0, in_=ip[:, 0:512])
                    nc.vector.tensor_copy(out=c1A, in_=ip[:, 512:1024])
                    nc.vector.tensor_copy(out=c1B, in_=ip[:, 1024:1536])
                st["c0"], st["c1"] = c0, [c1A, c1B]
                st["h0"], st["h1"] = None, [None, None]

            def l0_mms(st, t):
                s = st["s"]
                xb = 0 if t < 10 else 1024
                G0 = ppg[s].tile([128, 2048], f32, tag="G", name="G0")
                for gi in range(4):
                    reg = G0[:, gi * 512:(gi + 1) * 512]
                    for h in range(2):
                        nc.tensor.matmul(
                            G0[h * 64:(h + 1) * 64, gi * 512:(gi + 1) * 512],
                            w0pad[:, t, gi * 64:(gi + 1) * 64],
                            st["xT"][:, xb + h * 512:xb + (h + 1) * 512],
                            start=True, stop=(t == 0), skip_group_check=True)
                    if t > 0:
                        nc.tensor.matmul(reg, w0hh[:, gi * 128:(gi + 1) * 128],
                                         st["h0"], start=False, stop=True,
                                         skip_group_check=True)
                if has_gate_bias:
                    for gi in range(4):
                        nc.vector.tensor_scalar_add(
                            G0[:, gi * 512:(gi + 1) * 512],
                            G0[:, gi * 512:(gi + 1) * 512], gbias[:, gi:gi + 1])
                return G0

            def l1_mms(st, t, hf):
                s = st["s"]
                h0h = st["h0"][hf * 64:(hf + 1) * 64, :]
                w1s = w1dup[hf * 64:(hf + 1) * 64, :]
                G1 = ppg[s].tile([128, 2048], f32, tag="G", name="G1")
                for ci in range(4):
                    reg = G1[:, ci * 512:(ci + 1) * 512]
                    nc.tensor.matmul(reg, w1s[:, ci * 128:(ci + 1) * 128],
                                     h0h, start=True, stop=(t == 0))
                    if t > 0:
                        nc.tensor.matmul(reg, w1hh[:, ci * 128:(ci + 1) * 128],
                                         st["h1"][hf], start=False, stop=True)
                if has_gate_bias:
                    for ci in range(4):
                        nc.vector.tensor_scalar_add(
                            G1[:, ci * 512:(ci + 1) * 512],
                            G1[:, ci * 512:(ci + 1) * 512],
                            gbias[:, 4 + ci:5 + ci])
                return G1

            def pw_part1(st, G, c_st, sfx):
                # sig/gt on ACT; t1/t2/cres on DVE
                s = st["s"]
                sig = scratch.tile([128, 1536], bf16, tag=f"sig{sfx}{s}",
                                   name="sig")
                nc.scalar.activation(sig, G[:, 0:1536], AF.Sigmoid)
                gt = scratch.tile([128, NT], bf16, tag=f"gt{sfx}{s}", name="gt")
                nc.scalar.activation(gt, G[:, 1536:2048], AF.Tanh)
                t1 = scratch.tile([128, NT], bf16, tag=f"t1{sfx}{s}", name="t1")
                nc.vector.tensor_mul(t1, sig[:, 512:1024], c_st)
                t2 = scratch.tile([128, NT], bf16, tag=f"t2{sfx}{s}", name="t2")
                nc.vector.tensor_mul(t2, sig[:, 0:512], gt)
                cres = scratch.tile([128, NT], bf16, tag=f"cres{sfx}{s}",
                                    name="cres")
                nc.vector.tensor_add(cres, t1, t2)
                return sig, cres

            def pw_part2(st, sig, cres, c_st, sfx, c_tag, h_tag, h_rows):
                s = st["s"]
                cn = states.tile([h_rows, NT], f32, tag=f"{c_tag}{s}", name="cn")
                nc.vector.tensor_add(cn, c_st, cres)
                tc_ = scratch.tile([128, NT], bf16, tag=f"tc{sfx}{s}", name="tc_")
                nc.scalar.activation(tc_, cres, AF.Tanh)
                hn = states.tile([h_rows, NT], bf16, tag=f"{h_tag}{s}", name="hn")
                nc.vector.tensor_mul(hn, sig[:, 1024:1536], tc_)
                return hn, cn

            def head_stream(st, it):
                s = st["s"]
                h1 = st["h1"]
                hp = ppg[s].tile([128, 2048], f32, tag="G", name="hp")
                for hf in range(2):
                    nc.tensor.matmul(hp[0:H1, hf * 512:(hf + 1) * 512], d1w,
                                     h1[hf], start=True, stop=True)
                    z = outp.tile([H1, NT], bf16, tag=f"z{s}", name="z")
                    if has_vec_bias:
                        nc.vector.tensor_scalar_add(
                            z, hp[0:H1, hf * 512:(hf + 1) * 512], vbias[0:H1, 2:3])
                    else:
                        nc.vector.tensor_copy(
                            out=z, in_=hp[0:H1, hf * 512:(hf + 1) * 512])
                    nc.tensor.matmul(hp[0:1, 1024 + hf * 512:1024 + (hf + 1) * 512],
                                     d2w, z, start=True, stop=True)
                    out_sb = outp.tile([1, NT], f32, tag=f"out_sb{s}",
                                       name="out_sb")
                    if has_vec_bias:
                        nc.vector.tensor_scalar_add(
                            out_sb, hp[0:1, 1024 + hf * 512:1024 + (hf + 1) * 512],
                            vbias[0:1, 3:4])
                    else:
                        nc.vector.tensor_copy(
                            out=out_sb,
                            in_=hp[0:1, 1024 + hf * 512:1024 + (hf + 1) * 512])
                    # store on the ACT hwdge queue: keeps the SP queue free of
                    # head-dependent DMAs (no HOL block of next loads)
                    nc.scalar.dma_start(out=pred_view[it][hf], in_=out_sb)

            def phase_prep(q0):
                sts = [load_stream(0, q0), load_stream(1, q0 + 1)]
                for st in sts:
                    transpose_stream(st)
                    init_stream(st)
                return sts

            def phase_scan(sts):
                for t in range(T):
                    Gs = [l0_mms(st, t) for st in sts]
                    p1 = [pw_part1(st, G, st["c0"], "0")
                          for st, G in zip(sts, Gs)]
                    for st, (sig, cres) in zip(sts, p1):
                        hn, cn = pw_part2(st, sig, cres, st["c0"], "0",
                                          "c0", "h0", 128)
                        st["h0"], st["c0"] = hn, cn
                    for hf in range(2):
                        Gs = [l1_mms(st, t, hf) for st in sts]
                        p1 = [pw_part1(st, G, st["c1"][hf], "1")
                              for st, G in zip(sts, Gs)]
                        for st, (sig, cres) in zip(sts, p1):
                            hn, cn = pw_part2(st, sig, cres, st["c1"][hf],
                                              "1", f"c1{'AB'[hf]}",
                                              f"h1{'AB'[hf]}", H2)
                            st["h1"][hf], st["c1"][hf] = hn, cn

            def phase_finish(sts, it, q0):
                head_stream(sts[0], it + q0)
                head_stream(sts[1], it + q0 + 1)
                prefetch_stream(q0, it + q0)
                prefetch_stream(q0 + 1, it + q0 + 1)

            def pair_body(it):
                # two phases of two interleaved pair-tile streams. Phase B prep
                # is issued BEFORE phase A's head so B's transposes/init drain
                # on PE/DVE while A's last pointwise chains finish, instead of
                # queueing behind the head (which waits on A's final h1).
                stsA = phase_prep(0)
                phase_scan(stsA)
                stsB = phase_prep(2)
                phase_finish(stsA, it, 0)
                phase_scan(stsB)
                phase_finish(stsB, it, 2)

            hints = (nc.tensor.engine, nc.vector.engine, nc.scalar.engine,
                     nc.gpsimd.engine)
            if repeat == 1:
                prologue_loads()
                with tc.For_i(0, NPAIR, 4, hint_engines=hints,
                              staggered_reset=True) as it:
                    pair_body(it)
            else:  # benchmark variant: run the whole workload `repeat` times
                with tc.For_i(0, repeat, 1) as _r:
                    prologue_loads()
                    with tc.For_i(0, NPAIR, 4, hint_engines=hints,
                                  staggered_reset=True) as it:
                        pair_body(it)

    nc.finalize()
    return nc


def _get_nc(key):
    if key not in _BUILD_CACHE:
        _BUILD_CACHE[key] = _build_bass(*key)
    return _BUILD_CACHE[key]


def _prep_weights(inputs):
    # gate order permutation i,f,g,o -> i,f,o,g (sigmoid gates contiguous)
    def perm(n):
        return np.concatenate([np.arange(0, 2 * n), np.arange(3 * n, 4 * n),
                               np.arange(2 * n, 3 * n)])
    p0, p1 = perm(H1), perm(H2)

    w0ihT = inputs["l0_w_ih"][p0].T.astype(np.float32)     # [12, 256]
    w0hhT = inputs["l0_w_hh"][p0].T.astype(np.float32)     # [64, 256]
    w1ihT = inputs["l1_w_ih"][p1].T.astype(np.float32)     # [64, 512]
    w1hhT = inputs["l1_w_hh"][p1].T.astype(np.float32)     # [128, 512]

    # L0 ih: zero-padded full-K lhsT per step. xT_a rows are (t,f) elements
    # 0:128 (t0..t9), xT_b rows are elements 52:180 (t10..t14 at 12t-52).
    w0pad = np.zeros((128, T, 4 * H1), np.float32)
    for t in range(T):
        r0 = 12 * t if t < 10 else 12 * t - 52
        w0pad[r0:r0 + 12, t, :] = w0ihT
    # L0 hh block-diagonal (A rows 0:64, B rows 64:128)
    w0hh_bd = np.zeros((2 * H1, 512), np.float32)
    for g in range(4):
        blk = w0hhT[:, g * 64:(g + 1) * 64]
        w0hh_bd[0:64, g * 128:g * 128 + 64] = blk
        w0hh_bd[64:128, g * 128 + 64:(g + 1) * 128] = blk
    fc1T = inputs["fc1_w"].T.astype(np.float32)            # [96, 64]
    fc1_A = np.concatenate([fc1T, np.zeros_like(fc1T)], axis=1)      # [96, 128]
    fc1_B = np.concatenate([np.zeros_like(fc1T), fc1T], axis=1)

    wm = {
        "w0pad": w0pad.reshape(128, T * 4 * H1).astype(BF16),
        "w0hh_bd": w0hh_bd.astype(BF16),
        "w1dup": np.concatenate([w1ihT, w1ihT], axis=0).astype(BF16),
        "w1hhT": np.ascontiguousarray(w1hhT).astype(BF16),
        "fc1_A": fc1_A,
        "fc1_B": fc1_B,
        "fc2T": np.ascontiguousarray(inputs["fc2_w"].T).astype(np.float32),
        "d1T": np.ascontiguousarray(inputs["d1_w"].T).astype(BF16),
        "d2T": np.ascontiguousarray(inputs["d2_w"].T).astype(BF16),
    }

    b0 = (inputs["l0_b_ih"] + inputs["l0_b_hh"]).astype(np.float32)[p0]   # [256]
    b1 = (inputs["l1_b_ih"] + inputs["l1_b_hh"]).astype(np.float32)[p1]   # [512]
    gb = np.zeros((128, 8), np.float32)
    for g in range(4):
        gb[:, g] = np.tile(b0[g * 64:(g + 1) * 64], 2)     # stacked [A;B]
        gb[:, 4 + g] = b1[g * 128:(g + 1) * 128]
    vb = np.zeros((128, 4), np.float32)
    vb[:, 0] = np.tile(inputs["fc1_b"], 2)
    vb[:, 1] = inputs["fc2_b"]
    vb[0:H1, 2] = inputs["d1_b"]
    vb[0:1, 3] = inputs["d2_b"]
    wm["gate_bias"] = gb
    wm["vec_bias"] = vb
    has_gate_bias = bool(np.any(b0) or np.any(b1))
    has_vec_bias = bool(np.any(vb))
    return wm, has_gate_bias, has_vec_bias


def _in_maps(inputs, wm):
    x = inputs["input_seq"].astype(np.float32, copy=False)
    cs = inputs["cell_state"].astype(np.float32, copy=False)
    maps = []
    for i in range(NCORES):
        m = dict(wm)
        m["input_seq"] = np.ascontiguousarray(x[i * BL:(i + 1) * BL])
        m["cell_state"] = np.ascontiguousarray(cs[i * BL:(i + 1) * BL])
        maps.append(m)
    return maps


def kernel(**inputs):
    inputs = {k: np.asarray(v) for k, v in inputs.items()}
    wm, hgb, hvb = _prep_weights(inputs)
    nc = _get_nc((hgb, hvb))
    from concourse.bass_utils import run_bass_kernel_spmd
    res = run_bass_kernel_spmd(nc, _in_maps(inputs, wm),
                               core_ids=list(range(NCORES)))
    return np.concatenate([r["pred"] for r in res.results], axis=0)
